# revision 12
# baseline (speedup 1.0000x reference)
"""CFConv via PE-select: zero per-edge DMA. 8 TRN2 cores, dest-sharded.

Per core (12500 dest nodes, ~80k edges):
- x kept SBUF-resident one HALF (391 src tiles) at a time, node-major bf16.
- Edge stream sorted (dest group g of 12 blocks, src tile s); per (g,s) cell
  padded to max count over cores (SPMD-identical structure).
- Selection: one matmul per cell: lhsT = x_s [n,128in] (stationary),
  rhs = dense one-hot S [n, w cols] (uploaded bf16) -> xselT [in, w] PSUM,
  accumulated window-wise ([128,512] banks).
- Per 512-col window: xwT = W1.T @ xsel (+b1 via ACT copy-bias), filter MLP
  feature-major (hT = silu(Wf1.T@rbfT+bf1), filtT = Wf2.T@hT + bf2 via PSUM
  preload), msgT = filtT * xwT (DVE).
- Per tile: PE-transpose msgT -> msg [e,f]; dest one-hot Sd [e,1536] built
  on gpsimd (is_equal vs iota1536); scatter: 3 matmuls accumulate
  acc_k[f, 512] over the (g,h) segment's tiles.
- h-major: half 0 partials stored bf16 in SBUF; half 1 adds and finalizes
  out = acc.T @ W2 + b2 per block.
"""
import sys
sys.path.insert(0, '/opt/trn_rl_repo')
from contextlib import ExitStack

import ml_dtypes
import numpy as np

import concourse.bass as bass
import concourse.bacc as bacc
import concourse.tile as tile
from concourse import mybir
from concourse.bass_utils import run_bass_kernel_spmd

N_NODES = 100000
N_EDGES = 640000
D = 128
RBF = 16
NCORES = 8
NPC = N_NODES // NCORES          # 12500
NS = (N_NODES + 127) // 128      # 782 src tiles
NSH = (NS + 1) // 2              # 391 src tiles per half
GB = 12                          # dest blocks per group
GN = GB * 128                    # 1536 dest slots per group
NG = (NPC + GN - 1) // GN        # 9 groups (g8: 212 dests)
WIN = 512                        # selection window cols (1 PSUM bank)

F32 = mybir.dt.float32
BF16 = mybir.dt.bfloat16


def _preprocess(edge_index, edge_rbf):
    row = np.asarray(edge_index[0], dtype=np.int64)
    col = np.asarray(edge_index[1], dtype=np.int64)
    rbf = np.asarray(edge_rbf, dtype=np.float32)

    core = row // NPC
    pc = []
    for c in range(NCORES):
        m = core == c
        r = row[m] - c * NPC
        cl = col[m]
        g = r // GN
        s = cl // 128
        h = s // NSH
        order = np.lexsort((s, h, g))
        pc.append((r[order], cl[order], g[order], s[order], h[order],
                   rbf[m][order]))

    # cell widths: max count over cores per (g, s)
    cnt = np.zeros((NCORES, NG, NS), dtype=np.int64)
    for c in range(NCORES):
        _, _, g, s, _, _ = pc[c]
        np.add.at(cnt[c], (g, s), 1)
    width = cnt.max(axis=0)                    # [NG, NS]

    # build stream layout: segments (g, h); cells in s order within segment
    segs = []       # per (g,h): dict(start, ncols, cells=[(s, rel0, w)])
    pos = 0
    for g in range(NG):
        for h in range(2):
            start = pos
            cells = []
            s_lo, s_hi = h * NSH, min((h + 1) * NSH, NS)
            rel = 0
            for s in range(s_lo, s_hi):
                w = int(width[g, s])
                if w == 0:
                    continue
                cells.append([s, rel, w])
                rel += w
            # pad segment to a multiple of WIN by widening the last cell
            ncols = ((rel + WIN - 1) // WIN) * WIN
            if ncols == 0:
                cells.append([s_lo, 0, WIN])
                ncols = WIN
            elif ncols > rel:
                cells[-1][2] += ncols - rel
            segs.append(dict(g=g, h=h, start=start, ncols=ncols,
                             cells=cells))
            pos += ncols
    STREAM = pos
    T = STREAM // 128

    # cell start positions in the stream
    cell_start = np.full((NG, NS), -1, dtype=np.int64)
    for seg in segs:
        for (ss, rel0, w) in seg["cells"]:
            if cell_start[seg["g"], ss] < 0:
                cell_start[seg["g"], ss] = seg["start"] + rel0

    # per-core data arrays in stream order (vectorized placement)
    cores = []
    for c in range(NCORES):
        r, cl, g, s, h, rb = pc[c]
        E = len(r)
        key = g * NS + s
        first = np.r_[0, np.flatnonzero(np.diff(key)) + 1]
        runlen = np.diff(np.r_[first, E])
        rank = np.arange(E) - np.repeat(first, runlen)
        j = cell_start[g, s] + rank
        S_dense = np.zeros((128, STREAM), dtype=ml_dtypes.bfloat16)
        S_dense[cl % 128, j] = 1.0
        rbfT = np.zeros((RBF, STREAM), dtype=ml_dtypes.bfloat16)
        rbfT[:, j] = rb.T
        dloc = np.full((STREAM,), 4095.0, dtype=np.float32)
        dloc[j] = (r - g * GN).astype(np.float32)
        cores.append({
            "S": S_dense,
            "rbfT": rbfT,
            "dloc": dloc.reshape(T, 128).T.copy(),   # [128, T]
        })
    meta = dict(segs=segs, STREAM=STREAM, T=T)
    return cores, meta


def _build_program(meta, reps=1):
    segs, STREAM, T = meta["segs"], meta["STREAM"], meta["T"]

    nc = bacc.Bacc("TRN2", target_bir_lowering=False, debug=False,
                   num_devices=NCORES)

    xN_in = nc.dram_tensor("xN", [128, NS * 128], BF16,
                           kind="ExternalInput").ap()
    S_in = nc.dram_tensor("S", [128, STREAM], BF16, kind="ExternalInput").ap()
    rbf_in = nc.dram_tensor("rbfT", [RBF, STREAM], BF16,
                            kind="ExternalInput").ap()
    dloc_in = nc.dram_tensor("dloc", [128, T], F32, kind="ExternalInput").ap()
    W1_in = nc.dram_tensor("W1", [D, D], BF16, kind="ExternalInput").ap()
    Wf1_in = nc.dram_tensor("Wf1", [RBF, D], BF16, kind="ExternalInput").ap()
    Wf2_in = nc.dram_tensor("Wf2", [D, D], BF16, kind="ExternalInput").ap()
    W2_in = nc.dram_tensor("W2", [D, D], F32, kind="ExternalInput").ap()
    b1c_in = nc.dram_tensor("b1c", [128, 1], F32, kind="ExternalInput").ap()
    bf1c_in = nc.dram_tensor("bf1c", [128, 1], F32, kind="ExternalInput").ap()
    bf2c_in = nc.dram_tensor("bf2c", [128, 1], F32, kind="ExternalInput").ap()
    b2bc_in = nc.dram_tensor("b2bc", [128, D], F32, kind="ExternalInput").ap()
    iota_in = nc.dram_tensor("iota1536", [128, GN], F32,
                             kind="ExternalInput").ap()
    ident_in = nc.dram_tensor("ident", [128, 128], BF16,
                              kind="ExternalInput").ap()
    zc_in = nc.dram_tensor("zc", [128, 1], F32, kind="ExternalInput").ap()
    out_ext = nc.dram_tensor("out", [NPC, D], F32, kind="ExternalOutput").ap()

    with tile.TileContext(nc) as tc:
        with ExitStack() as ctx:
            res = ctx.enter_context(tc.tile_pool(name="res", bufs=1))
            xhp = ctx.enter_context(tc.tile_pool(name="xh", bufs=1))
            sp = ctx.enter_context(tc.tile_pool(name="schunk", bufs=2))
            rp = ctx.enter_context(tc.tile_pool(name="rchunk", bufs=2))
            selp = ctx.enter_context(tc.tile_pool(name="selsb", bufs=2))
            xwp = ctx.enter_context(tc.tile_pool(name="xwsb", bufs=2))
            hp = ctx.enter_context(tc.tile_pool(name="hsb", bufs=2))
            mp = ctx.enter_context(tc.tile_pool(name="msgt", bufs=2))
            ep = ctx.enter_context(tc.tile_pool(name="msge", bufs=3))
            sdp = ctx.enter_context(tc.tile_pool(name="sd", bufs=2))
            accp = ctx.enter_context(tc.tile_pool(name="accsb", bufs=9))
            finp = ctx.enter_context(tc.tile_pool(name="finsb", bufs=2))
            pp_sel = ctx.enter_context(
                tc.tile_pool(name="psel", bufs=2, space="PSUM"))
            pp_mm = ctx.enter_context(
                tc.tile_pool(name="pmm", bufs=2, space="PSUM"))
            pp_tr = ctx.enter_context(
                tc.tile_pool(name="ptr", bufs=1, space="PSUM"))
            pp_a = [ctx.enter_context(
                tc.tile_pool(name=f"pa{k}", bufs=1, space="PSUM"))
                for k in range(3)]

            W1 = res.tile([D, D], BF16); nc.sync.dma_start(W1[:], W1_in[:])
            Wf1 = res.tile([RBF, D], BF16)
            nc.sync.dma_start(Wf1[:], Wf1_in[:])
            Wf2 = res.tile([D, D], BF16); nc.sync.dma_start(Wf2[:], Wf2_in[:])
            W2 = res.tile([D, D], F32); nc.sync.dma_start(W2[:], W2_in[:])
            b1c = res.tile([128, 1], F32); nc.sync.dma_start(b1c[:], b1c_in[:])
            bf1c = res.tile([128, 1], F32)
            nc.sync.dma_start(bf1c[:], bf1c_in[:])
            bf2c = res.tile([128, 1], F32)
            nc.sync.dma_start(bf2c[:], bf2c_in[:])
            b2bc = res.tile([128, D], F32)
            nc.sync.dma_start(b2bc[:], b2bc_in[:])
            iota = res.tile([128, GN], F32)
            nc.sync.dma_start(iota[:], iota_in[:])
            ident = res.tile([128, 128], BF16)
            nc.sync.dma_start(ident[:], ident_in[:])
            zc = res.tile([128, 1], F32); nc.sync.dma_start(zc[:], zc_in[:])
            dloc = res.tile([128, T], F32)
            nc.sync.dma_start(dloc[:], dloc_in[:])

            for _rep in range(reps):
                partials = {}
                for h in range(2):
                    s_lo = h * NSH
                    nst = min(NSH, NS - s_lo)
                    xh = xhp.tile([128, NSH * 128], BF16, tag="xh")
                    nc.sync.dma_start(
                        xh[:, :nst * 128],
                        xN_in[:, s_lo * 128:(s_lo + nst) * 128])
                    for seg in segs:
                        if seg["h"] != h:
                            continue
                        g = seg["g"]
                        base, ncols = seg["start"], seg["ncols"]
                        nwin = ncols // WIN
                        ntile = ncols // 128
                        acc = [pp_a[k].tile([128, WIN], F32, tag=f"acc{k}",
                                            space="PSUM", name=f"acc{k}")
                               for k in range(3)]
                        first_t = True
                        # iterate windows; emit selection cells per window
                        cells = seg["cells"]
                        cidx = 0
                        coff = 0  # consumed cols of current cell
                        for wi in range(nwin):
                            w0 = wi * WIN      # segment-rel window start
                            sel_ps = pp_sel.tile([128, WIN], F32, tag="sel",
                                                 space="PSUM", name="sel")
                            Sch = sp.tile([128, WIN], BF16, tag="Sch")
                            nc.sync.dma_start(
                                Sch[:], S_in[:, base + w0:base + w0 + WIN])
                            rch = rp.tile([RBF, WIN], BF16, tag="rch")
                            nc.sync.dma_start(
                                rch[:], rbf_in[:, base + w0:base + w0 + WIN])
                            filled = 0
                            while filled < WIN:
                                s, rel0, w = cells[cidx]
                                c0 = rel0 + coff          # seg-rel col
                                avail = w - coff
                                take = min(avail, WIN - (c0 - w0))
                                rel_in_win = c0 - w0
                                nc.tensor.matmul(
                                    sel_ps[:, rel_in_win:rel_in_win + take],
                                    lhsT=xh[:, (s - s_lo) * 128:
                                            (s - s_lo + 1) * 128],
                                    rhs=Sch[:, rel_in_win:rel_in_win + take],
                                    start=True, stop=True)
                                filled = rel_in_win + take
                                if coff + take == w:
                                    cidx += 1
                                    coff = 0
                                else:
                                    coff += take
                            # window chain: xw, filter, modulate
                            sel_sb = selp.tile([128, WIN], BF16, tag="selsb")
                            nc.scalar.activation(
                                sel_sb[:], sel_ps[:],
                                mybir.ActivationFunctionType.Copy,
                                bias=0.0, scale=1.0)
                            xw_ps = pp_mm.tile([128, WIN], F32, tag="mmw",
                                               space="PSUM", name="xwps")
                            nc.scalar.activation(
                                xw_ps[:],
                                b1c[:, :1].to_broadcast([128, WIN]),
                                mybir.ActivationFunctionType.Copy,
                                bias=0.0, scale=1.0)
                            nc.tensor.matmul(xw_ps[:], lhsT=W1[:],
                                             rhs=sel_sb[:], start=False,
                                             stop=True)
                            xw_sb = xwp.tile([128, WIN], BF16, tag="xwsb")
                            nc.scalar.activation(
                                xw_sb[:], xw_ps[:],
                                mybir.ActivationFunctionType.Copy,
                                bias=0.0, scale=1.0)
                            h_ps = pp_mm.tile([128, WIN], F32, tag="mmw",
                                              space="PSUM", name="hps")
                            nc.tensor.matmul(h_ps[:], lhsT=Wf1[:],
                                             rhs=rch[:], start=True,
                                             stop=True)
                            h_sb = hp.tile([128, WIN], BF16, tag="hsb")
                            nc.scalar.activation(
                                h_sb[:], h_ps[:],
                                mybir.ActivationFunctionType.Silu,
                                bias=bf1c[:, :1], scale=1.0)
                            filt_ps = pp_mm.tile([128, WIN], F32, tag="mmw",
                                                 space="PSUM", name="filtps")
                            nc.scalar.activation(
                                filt_ps[:],
                                bf2c[:, :1].to_broadcast([128, WIN]),
                                mybir.ActivationFunctionType.Copy,
                                bias=0.0, scale=1.0)
                            nc.tensor.matmul(filt_ps[:], lhsT=Wf2[:],
                                             rhs=h_sb[:], start=False,
                                             stop=True)
                            msgT = mp.tile([128, WIN], BF16, tag="msgt")
                            nc.vector.tensor_mul(msgT[:], filt_ps[:],
                                                 xw_sb[:])
                            # per tile: transpose + dest one-hot + scatter
                            for k in range(4):
                                t_seg = wi * 4 + k
                                t_abs = (base // 128) + t_seg
                                tr_ps = pp_tr.tile([128, 128], BF16,
                                                   tag="tr", space="PSUM",
                                                   name="trps")
                                nc.tensor.transpose(
                                    tr_ps[:], msgT[:, k * 128:(k + 1) * 128],
                                    ident[:])
                                msg_e = ep.tile([128, 128], BF16, tag="msge")
                                nc.vector.tensor_copy(msg_e[:], tr_ps[:])
                                Sd = sdp.tile([128, GN], BF16, tag="sd")
                                nc.vector.tensor_tensor(
                                    out=Sd[:], in0=iota[:],
                                    in1=dloc[:, t_abs:t_abs + 1].to_broadcast(
                                        [128, GN]),
                                    op=mybir.AluOpType.is_equal)
                                last_t = t_seg == ntile - 1
                                for kk in range(3):
                                    nc.tensor.matmul(
                                        acc[kk][:],
                                        lhsT=msg_e[:],
                                        rhs=Sd[:, kk * WIN:(kk + 1) * WIN],
                                        start=first_t, stop=last_t)
                                first_t = False
                        # segment done: stash (h=0) or finalize (h=1)
                        if h == 0:
                            part = accp.tile([128, 3 * WIN], BF16, tag="part",
                                             name=f"part{g}")
                            for kk in range(3):
                                nc.vector.tensor_copy(
                                    part[:, kk * WIN:(kk + 1) * WIN],
                                    acc[kk][:])
                            partials[g] = part
                        else:
                            part = partials[g]
                            full = finp.tile([128, 3 * WIN], F32, tag="full")
                            for kk in range(3):
                                nc.vector.tensor_add(
                                    full[:, kk * WIN:(kk + 1) * WIN],
                                    acc[kk][:],
                                    part[:, kk * WIN:(kk + 1) * WIN])
                            nblk = min(GB, (NPC - g * GN + 127) // 128)
                            for b in range(nblk):
                                fin_ps = pp_tr.tile([128, 128], F32,
                                                    tag="tr", space="PSUM",
                                                    name="finps")
                                nc.tensor.matmul(
                                    fin_ps[:],
                                    lhsT=full[:, b * 128:(b + 1) * 128],
                                    rhs=W2[:], start=True, stop=True)
                                fin_sb = finp.tile([128, 128], F32,
                                                   tag="finsb")
                                nc.vector.tensor_add(fin_sb[:], fin_ps[:],
                                                     b2bc[:])
                                r0 = g * GN + b * 128
                                rows = min(128, NPC - r0)
                                nc.sync.dma_start(
                                    out_ext[r0:r0 + rows, :],
                                    fin_sb[:rows, :])
    nc.compile()
    return nc


def _make_in_maps(x, W1, b1, Wf1, bf1, Wf2, bf2, W2, b2, cores):
    xp = np.zeros((NS * 128, D), dtype=np.float32)
    xp[:N_NODES] = np.asarray(x, dtype=np.float32)
    # node-major: partition = node%128, col = s*128 + f
    xN = xp.reshape(NS, 128, D).transpose(1, 0, 2).reshape(128, NS * 128)
    common = {
        "xN": xN.astype(ml_dtypes.bfloat16),
        "W1": np.asarray(W1, np.float32).astype(ml_dtypes.bfloat16),
        "Wf1": np.asarray(Wf1, np.float32).astype(ml_dtypes.bfloat16),
        "Wf2": np.asarray(Wf2, np.float32).astype(ml_dtypes.bfloat16),
        "W2": np.asarray(W2, np.float32),
        "b1c": np.asarray(b1, np.float32).reshape(128, 1).copy(),
        "bf1c": np.asarray(bf1, np.float32).reshape(128, 1).copy(),
        "bf2c": np.asarray(bf2, np.float32).reshape(128, 1).copy(),
        "b2bc": np.broadcast_to(np.asarray(b2, np.float32), (128, D)).copy(),
        "iota1536": np.broadcast_to(np.arange(GN, dtype=np.float32),
                                    (128, GN)).copy(),
        "ident": np.eye(128, dtype=ml_dtypes.bfloat16),
        "zc": np.zeros((128, 1), np.float32),
    }
    in_maps = []
    for c in range(NCORES):
        m = dict(common)
        m["S"] = cores[c]["S"]
        m["rbfT"] = cores[c]["rbfT"]
        m["dloc"] = cores[c]["dloc"]
        in_maps.append(m)
    return in_maps


_CACHE = {}


def kernel(x, edge_index, edge_rbf, W1, b1, Wf1, bf1, Wf2, bf2, W2, b2):
    cores, meta = _preprocess(edge_index, edge_rbf)
    key = (meta["STREAM"],
           tuple(tuple(map(tuple, s["cells"])) for s in meta["segs"]))
    kh = hash(key)
    if kh not in _CACHE:
        _CACHE[kh] = _build_program(meta, reps=1)
    nc = _CACHE[kh]
    in_maps = _make_in_maps(x, W1, b1, Wf1, bf1, Wf2, bf2, W2, b2, cores)
    res = run_bass_kernel_spmd(nc, in_maps, list(range(NCORES)))
    out = np.concatenate([res.results[c]["out"] for c in range(NCORES)],
                         axis=0)
    return out.astype(np.float32)


# revision 13
# speedup vs baseline: 1.6236x; 1.6236x over previous
"""CFConv (SchNet continuous-filter conv) Bass kernel for 8 Trainium2 NeuronCores.

Strategy (graph/data parallel per the sharding hint):
- Nodes partitioned 12500/core; edges routed to the owner of their destination
  node (row) so the scatter-add is local to a core.
- Each core computes xw = x @ W1 + b1 for ALL nodes into an internal DRAM
  table (partition-striped: node k lives at row (k%128)*782 + k//128 of a
  [100096, 128] table, so phase-A writes are one descriptor per partition).
- Per-edge xw rows are fetched with gpsimd.dma_gather (the Q7 MoE gather:
  int16 indices wrapped in 16 partitions and replicated per Q7 core). int16
  limits a gather to a 25024-row window, so the table is split into 4
  quarters; an edge's quarter is (col%128)//32. Destination blocks of 128
  nodes are processed in supergroups of 4 blocks; edges are grouped
  (block, quarter) and padded to whole 128-edge tiles; one dma_gather per
  (supergroup, quarter) fetches all its tiles at once.
- Scatter-add is a matmul with a one-hot selection matrix S built on-device
  by an is_equal compare: outT[f, n] += msg[e, f].T @ S[e, n], accumulated in
  one PSUM bank per block (4 concurrent blocks).
- Filter MLP per up-to-4-tile window: h1T = Wf1.T @ rbfT (K=16), Silu(+bf1)
  on ACT, filt = hT_slice.T @ Wf2 per tile, msg = (filt + bf2) * xw_g on DVE.
- Per block: final = outT.T @ W2 + b2, DMA'd to the output rows.

SPMD: one program for all 8 cores; per-(core, block, quarter) edge counts are
padded to a common tile count (max over cores) so instruction streams match.
"""
import sys
sys.path.insert(0, '/opt/trn_rl_repo')
from contextlib import ExitStack

import ml_dtypes
import numpy as np

import concourse.bass as bass
import concourse.bacc as bacc
import concourse.tile as tile
from concourse import library_config, mybir
from concourse.bass_utils import run_bass_kernel_spmd

N_NODES = 100000
N_EDGES = 640000
D = 128
RBF = 16
NCORES = 8
NPC = N_NODES // NCORES        # 12500 nodes per core
BLK = 128
NBLK = (NPC + BLK - 1) // BLK  # 98 dest blocks per core
NTILES_X = (N_NODES + 127) // 128  # 782 node tiles in the xw table
NPAD = NTILES_X * 128          # 100096
NQ = 4
QROWS = NPAD // NQ             # 25024 table rows per quarter (int16-safe)
SGB = 4                        # blocks per supergroup
GRP = 4                        # edge tiles per filter-MLP window

F32 = mybir.dt.float32
BF16 = mybir.dt.bfloat16
I16 = mybir.dt.int16

assert QROWS == 32 * NTILES_X  # quarter of an edge == (col%128)//32


def _preprocess(edge_index, edge_rbf):
    """Route edges by dest owner; group by (dest block, table quarter); pad
    each group to whole 128-edge tiles with a common count across cores."""
    row = np.asarray(edge_index[0], dtype=np.int64)
    col = np.asarray(edge_index[1], dtype=np.int64)
    rbf = np.asarray(edge_rbf, dtype=np.float32)

    core = row // NPC
    per_core = []
    counts = np.zeros((NCORES, NBLK, NQ), dtype=np.int64)
    for c in range(NCORES):
        m = core == c
        r = row[m] - c * NPC
        cl = col[m]
        tr = (cl % 128) * NTILES_X + cl // 128   # striped table row
        # quarter-split so idx16 = tr - q*QROWS fits in int16 for dma_gather
        q = tr // QROWS
        b = r // BLK
        order = np.lexsort((q, b))
        r, cl, tr, q, b = r[order], cl[order], tr[order], q[order], b[order]
        rb = rbf[m][order]
        np.add.at(counts[c], (b, q), 1)
        per_core.append((r, tr, rb, b, q))

    tpbq = (counts.max(axis=0) + BLK - 1) // BLK          # [NBLK, NQ]
    for b in range(NBLK):
        if tpbq[b].sum() == 0:
            tpbq[b][0] = 1  # keep >=1 tile so outT is always written

    # tile stream: supergroups of SGB blocks; within one, quarter-major
    # (one dma_gather per (sg, q) needs its tiles contiguous)
    tiles = []          # (b, q) per tile
    ops_by_sg = []      # per supergroup: list of (q, tile_start, ntiles)
    for sg0 in range(0, NBLK, SGB):
        bs = range(sg0, min(sg0 + SGB, NBLK))
        sg_ops = []
        for q in range(NQ):
            nt = int(sum(tpbq[b][q] for b in bs))
            if nt == 0:
                continue
            sg_ops.append((q, len(tiles), nt))
            for b in bs:
                tiles.extend([(b, q)] * int(tpbq[b][q]))
        ops_by_sg.append(sg_ops)
    T = len(tiles)
    kmax = max(nt for sg in ops_by_sg for _, _, nt in sg)

    blk_first = {}
    blk_last = {}
    for t, (b, q) in enumerate(tiles):
        blk_first.setdefault(b, t)
        blk_last[b] = t

    # per-(b,q) tile start offsets in the stream
    seg_start = {}
    for t, (b, q) in enumerate(tiles):
        seg_start.setdefault((b, q), t)

    cores = []
    for c in range(NCORES):
        r, tr, rb, b, q = per_core[c]
        idx16 = np.zeros((T * BLK,), dtype=np.int16)
        idx32 = np.zeros((T * BLK,), dtype=np.int32)
        rl = np.full((T * BLK,), 255.0, dtype=np.float32)
        rbfT = np.zeros((RBF, T * BLK), dtype=np.float32)
        e0 = 0
        for bb in range(NBLK):
            for qq in range(NQ):
                n = int(counts[c, bb, qq])
                if n == 0:
                    continue
                d0 = seg_start[(bb, qq)] * BLK
                dst = slice(d0, d0 + n)
                idx16[dst] = (tr[e0:e0 + n] - qq * QROWS).astype(np.int16)
                idx32[dst] = tr[e0:e0 + n].astype(np.int32)
                rl[dst] = (r[e0:e0 + n] - bb * BLK).astype(np.float32)
                rbfT[:, dst] = rb[e0:e0 + n].T
                e0 += n
        # idx wrapped in 16 partitions, replicated for the 8 Q7 cores
        idxw = np.tile(idx16.reshape(T * 8, 16).T, (8, 1)).copy()
        cores.append({
            "idx16": idxw,                              # [128, T*8] int16
            "idxp": idx32.reshape(T, BLK).T.copy(),     # [128, T] int32
            "rl": rl.reshape(T, BLK).T.copy(),          # [128, T]
            "rbfT": rbfT,                               # [16, T*128]
        })
    meta = dict(tiles=tiles, ops_by_sg=ops_by_sg, kmax=int(kmax),
                blk_first=blk_first, blk_last=blk_last)
    return cores, tpbq, T, meta


def _build_program(T, meta, reps=1, sim_mode=False, do_phase_a=True,
                   do_phase_b=True, do_gather=True, do_compute=True,
                   use_dma_gather=True):
    tiles, ops_by_sg, kmax = meta["tiles"], meta["ops_by_sg"], meta["kmax"]
    blk_first, blk_last = meta["blk_first"], meta["blk_last"]

    nc = bacc.Bacc("TRN2", target_bir_lowering=False, debug=False,
                   num_devices=NCORES)

    xT_in = nc.dram_tensor("xT", [128, NPAD], F32, kind="ExternalInput").ap()
    W1_in = nc.dram_tensor("W1", [D, D], F32, kind="ExternalInput").ap()
    Wf1_in = nc.dram_tensor("Wf1", [RBF, D], F32, kind="ExternalInput").ap()
    Wf2_in = nc.dram_tensor("Wf2", [D, D], F32, kind="ExternalInput").ap()
    W2_in = nc.dram_tensor("W2", [D, D], F32, kind="ExternalInput").ap()
    b1bc_in = nc.dram_tensor("b1bc", [128, D], F32, kind="ExternalInput").ap()
    bf1c_in = nc.dram_tensor("bf1c", [128, 1], F32, kind="ExternalInput").ap()
    bf2bc_in = nc.dram_tensor("bf2bc", [128, D], F32, kind="ExternalInput").ap()
    b2bc_in = nc.dram_tensor("b2bc", [128, D], F32, kind="ExternalInput").ap()
    iota_in = nc.dram_tensor("iotar", [128, 128], F32, kind="ExternalInput").ap()
    idx_in = nc.dram_tensor("idx16", [128, T * 8], I16, kind="ExternalInput").ap()
    idxp_in = nc.dram_tensor("idxp", [128, T], mybir.dt.int32, kind="ExternalInput").ap()
    rl_in = nc.dram_tensor("rl", [128, T], F32, kind="ExternalInput").ap()
    rbfT_in = nc.dram_tensor("rbfT", [RBF, T * BLK], F32, kind="ExternalInput").ap()
    out_ext = nc.dram_tensor("out", [NPC, D], F32, kind="ExternalOutput").ap()

    # internal xw table, partition-striped: node k at [k%128, (k//128)*128+f].
    # Four naturally-shaped [QROWS, 128] quarter tensors (dma_gather wants a
    # plain [rows, elem] table); quarter q holds partitions [32q, 32q+32) of
    # the [128, NPAD] striped view. Writes go through a manual AP per quarter.
    xwq_h = [nc.dram_tensor(f"xw{q}", [QROWS, 128], F32, kind="ExternalOutput")
             for q in range(NQ)] if use_dma_gather else []
    xw_q = [h.ap() for h in xwq_h]
    # [32-partition, NPAD] write views (partition j of quarter q = global
    # partition 32q+j; its row-range is [j*NTILES_X, (j+1)*NTILES_X))
    xwq_w = [bass.AP(xw_q[q].tensor, 0, [[NPAD, 32], [1, NPAD]])
             for q in range(len(xw_q))]
    xw_tab = None  # indirect path keeps a single full-table tensor
    xw_h = nc.dram_tensor("xw", [128, NPAD], F32)
    xw_w = xw_h.ap()
    if not use_dma_gather:
        xw_tab = bass.AP(xw_w.tensor, 0, [[128, NPAD], [1, 128]])

    if not sim_mode:
        nc.gpsimd.load_library(library_config.mlp)

    XCH = 32
    NCH = (NTILES_X + XCH - 1) // XCH
    gsem = nc.alloc_semaphore("gsem")
    gcount = [0]

    with tile.TileContext(nc) as tc:
        with ExitStack() as ctx:
            res = ctx.enter_context(tc.tile_pool(name="res", bufs=1))
            xpool = ctx.enter_context(tc.tile_pool(name="xch", bufs=2))
            spool = ctx.enter_context(tc.tile_pool(name="stage", bufs=2))
            gpool = ctx.enter_context(tc.tile_pool(name="gath", bufs=2))
            rpool = ctx.enter_context(tc.tile_pool(name="rbfp", bufs=2))
            wpool = ctx.enter_context(tc.tile_pool(name="work", bufs=3))
            hpool = ctx.enter_context(tc.tile_pool(name="hts", bufs=3))
            opool = ctx.enter_context(tc.tile_pool(name="outs", bufs=2))
            pp_mm = ctx.enter_context(tc.tile_pool(name="psmm", bufs=2, space="PSUM"))
            pp_h1 = ctx.enter_context(tc.tile_pool(name="psh1", bufs=1, space="PSUM"))
            pp_out = ctx.enter_context(tc.tile_pool(name="psout", bufs=4, space="PSUM"))
            pp_fin = ctx.enter_context(tc.tile_pool(name="psfin", bufs=1, space="PSUM"))

            W1 = res.tile([D, D], F32); nc.sync.dma_start(W1[:], W1_in[:])
            Wf1 = res.tile([RBF, D], F32); nc.sync.dma_start(Wf1[:], Wf1_in[:])
            Wf2 = res.tile([D, D], F32); nc.sync.dma_start(Wf2[:], Wf2_in[:])
            W2 = res.tile([D, D], F32); nc.sync.dma_start(W2[:], W2_in[:])
            b1bc = res.tile([128, D], F32); nc.sync.dma_start(b1bc[:], b1bc_in[:])
            bf1c = res.tile([128, 1], F32); nc.sync.dma_start(bf1c[:], bf1c_in[:])
            bf2bc = res.tile([128, D], F32); nc.sync.dma_start(bf2bc[:], bf2bc_in[:])
            b2bc = res.tile([128, D], F32); nc.sync.dma_start(b2bc[:], b2bc_in[:])
            iotar = res.tile([128, 128], F32); nc.sync.dma_start(iotar[:], iota_in[:])
            idx_sb = res.tile([128, T * 8], I16); nc.sync.dma_start(idx_sb[:], idx_in[:])
            idxp = res.tile([128, T], mybir.dt.int32); nc.sync.dma_start(idxp[:], idxp_in[:])
            rl = res.tile([128, T], F32); nc.sync.dma_start(rl[:], rl_in[:])
            # token tile: every gather critical-unit writes it, forcing Tile
            # to keep the units in emission order on the gpsimd stream (the
            # cumulative gsem waits rely on that order). xwtap is a dummy
            # Tile-visible read of the xw tensor for phase-A -> B ordering.
            token = res.tile([1, 8], F32)
            xwtap = res.tile([1, 8], F32)

            for _rep in range(reps):
                # ---- phase A: xw = x @ W1 + b1 for all nodes ----
                for ch in range(NCH if do_phase_a else 0):
                    i0 = ch * XCH
                    nt = min(XCH, NTILES_X - i0)
                    xch = xpool.tile([128, XCH * 128], F32, tag="xch")
                    nc.sync.dma_start(xch[:, :nt * 128],
                                      xT_in[:, i0 * 128:(i0 + nt) * 128])
                    stage = spool.tile([128, XCH * 128], F32, tag="stage")
                    for i in range(nt):
                        xw_ps = pp_mm.tile([128, 128], F32, tag="mm128",
                                           space="PSUM")
                        nc.tensor.matmul(xw_ps[:],
                                         lhsT=xch[:, i * 128:(i + 1) * 128],
                                         rhs=W1[:], start=True, stop=True)
                        nc.vector.tensor_add(stage[:, i * 128:(i + 1) * 128],
                                             xw_ps[:], b1bc[:])
                    if use_dma_gather:
                        c0, c1 = i0 * 128, (i0 + nt) * 128
                        for q in range(NQ):
                            wv = bass.AP(xw_q[q].tensor, c0,
                                         [[NPAD, 32], [1, c1 - c0]])
                            nc.sync.dma_start(wv,
                                              stage[32 * q:32 * (q + 1),
                                                    :nt * 128])
                    else:
                        nc.sync.dma_start(xw_w[:, i0 * 128:(i0 + nt) * 128],
                                          stage[:, :nt * 128])

                # ---- phase B: edges, one gather per (supergroup, quarter) ----
                if do_phase_b:
                    outT = {}
                    for sgi, sg0 in enumerate(range(0, NBLK, SGB)):
                        bs = list(range(sg0, min(sg0 + SGB, NBLK)))
                        for b in bs:
                            outT[b] = pp_out.tile([128, 128], F32, tag="outT",
                                                  space="PSUM", name=f"oT{b}")
                        for q, t0, ntq in ops_by_sg[sgi]:
                            xwg = None
                            if do_gather or do_compute:
                                xwg = gpool.tile([128, kmax, 128], F32,
                                                 tag="xwg", name="xwg")
                            if do_gather and use_dma_gather:
                                # the SWDGE descriptor ring holds 1024
                                # descriptors; one dma_gather op must stay
                                # <= 1024 indices (8 tiles) or the Q7
                                # handler crashes the NEFF.
                                for c0 in range(0, ntq, 8):
                                    csz = min(8, ntq - c0)
                                    nc.gpsimd.dma_gather(
                                        out_ap=xwg[:, c0:c0 + csz, :],
                                        in_ap=xw_q[q],
                                        idxs_ap=idx_sb[:, (t0 + c0) * 8:
                                                       (t0 + c0 + csz) * 8],
                                        num_idxs=csz * BLK,
                                        num_idxs_reg=csz * BLK,
                                        elem_size=D)
                            rbft = rpool.tile([RBF, kmax * BLK], F32, tag="rbf")
                            nc.sync.dma_start(
                                rbft[:, :ntq * BLK],
                                rbfT_in[:, t0 * BLK:(t0 + ntq) * BLK])
                            if not do_compute:
                                continue
                            for g0 in range(0, ntq, GRP):
                                gsz = min(GRP, ntq - g0)
                                h1 = pp_h1.tile([128, GRP * BLK], F32,
                                                tag="h1", space="PSUM")
                                nc.tensor.matmul(
                                    h1[:, :gsz * BLK], lhsT=Wf1[:],
                                    rhs=rbft[:, g0 * BLK:(g0 + gsz) * BLK],
                                    start=True, stop=True)
                                hT = hpool.tile([128, GRP * BLK], F32, tag="hT")
                                if sim_mode:
                                    sg_t = hpool.tile([128, GRP * BLK], F32,
                                                      tag="sg")
                                    nc.scalar.activation(
                                        sg_t[:, :gsz * BLK], h1[:, :gsz * BLK],
                                        mybir.ActivationFunctionType.Sigmoid,
                                        bias=bf1c[:, :1], scale=1.0)
                                    zz = hpool.tile([128, GRP * BLK], F32,
                                                    tag="zz")
                                    nc.vector.tensor_scalar(
                                        out=zz[:, :gsz * BLK],
                                        in0=h1[:, :gsz * BLK],
                                        scalar1=bf1c[:, :1], scalar2=None,
                                        op0=mybir.AluOpType.add)
                                    nc.vector.tensor_mul(hT[:, :gsz * BLK],
                                                         zz[:, :gsz * BLK],
                                                         sg_t[:, :gsz * BLK])
                                else:
                                    nc.scalar.activation(
                                        hT[:, :gsz * BLK], h1[:, :gsz * BLK],
                                        mybir.ActivationFunctionType.Silu,
                                        bias=bf1c[:, :1], scale=1.0)
                                for i in range(gsz):
                                    t = t0 + g0 + i
                                    b = tiles[t][0]
                                    if not use_dma_gather and do_gather:
                                        xwg_t = gpool.tile([128, 128], F32,
                                                           tag="xwgt")
                                        nc.gpsimd.indirect_dma_start(
                                            out=xwg_t[:], out_offset=None,
                                            in_=xw_tab,
                                            in_offset=bass.IndirectOffsetOnAxis(
                                                ap=idxp[:, t:t + 1], axis=0))
                                        xsrc = xwg_t[:]
                                    else:
                                        xsrc = xwg[:, g0 + i, :]
                                    S = wpool.tile([128, 128], F32, tag="S")
                                    nc.vector.tensor_tensor(
                                        out=S[:], in0=iotar[:],
                                        in1=rl[:, t:t + 1].to_broadcast(
                                            [128, 128]),
                                        op=mybir.AluOpType.is_equal)
                                    filt_ps = pp_mm.tile([128, 128], F32,
                                                         tag="mm128",
                                                         space="PSUM")
                                    nc.tensor.matmul(
                                        filt_ps[:],
                                        lhsT=hT[:, i * 128:(i + 1) * 128],
                                        rhs=Wf2[:], start=True, stop=True)
                                    msg = wpool.tile([128, 128], F32, tag="msg")
                                    nc.vector.scalar_tensor_tensor(
                                        out=msg[:], in0=filt_ps[:], scalar=1.0,
                                        in1=bf2bc[:], op0=mybir.AluOpType.mult,
                                        op1=mybir.AluOpType.add)
                                    msgm = wpool.tile([128, 128], F32,
                                                      tag="msgm")
                                    nc.vector.tensor_mul(msgm[:], msg[:],
                                                         xsrc)
                                    nc.tensor.matmul(
                                        outT[b][:], lhsT=msgm[:], rhs=S[:],
                                        start=(t == blk_first[b]),
                                        stop=(t == blk_last[b]))
                        if not do_compute:
                            continue
                        for b in bs:
                            outT_sb = opool.tile([128, 128], F32, tag="outTsb")
                            nc.vector.tensor_copy(outT_sb[:], outT[b][:])
                            fin_ps = pp_fin.tile([128, 128], F32, tag="fin",
                                                 space="PSUM")
                            nc.tensor.matmul(fin_ps[:], lhsT=outT_sb[:],
                                             rhs=W2[:], start=True, stop=True)
                            fin = opool.tile([128, 128], F32, tag="fin_sb")
                            nc.vector.tensor_add(fin[:], fin_ps[:], b2bc[:])
                            rows = min(BLK, NPC - b * BLK)
                            nc.sync.dma_start(
                                out_ext[b * BLK:b * BLK + rows, :],
                                fin[:rows, :])
    nc.compile()
    return nc


def _make_in_maps(x, edge_index, edge_rbf, W1, b1, Wf1, bf1, Wf2, bf2, W2, b2,
                  cores, T):
    xT = np.zeros((128, NPAD), dtype=np.float32)
    xp = np.zeros((NPAD, D), dtype=np.float32)
    xp[:N_NODES] = np.asarray(x, dtype=np.float32)
    # xT[:, i*128:(i+1)*128] is node-tile i, feature-on-partition
    xT[:] = xp.reshape(NTILES_X, 128, D).transpose(2, 0, 1).reshape(D, NPAD)

    common = {
        "xT": xT,
        "W1": np.asarray(W1, np.float32),
        "Wf1": np.asarray(Wf1, np.float32),
        "Wf2": np.asarray(Wf2, np.float32),
        "W2": np.asarray(W2, np.float32),
        "b1bc": np.broadcast_to(np.asarray(b1, np.float32), (128, D)).copy(),
        "bf1c": np.asarray(bf1, np.float32).reshape(128, 1).copy(),
        "bf2bc": np.broadcast_to(np.asarray(bf2, np.float32), (128, D)).copy(),
        "b2bc": np.broadcast_to(np.asarray(b2, np.float32), (128, D)).copy(),
        "iotar": np.broadcast_to(np.arange(128, dtype=np.float32),
                                 (128, 128)).copy(),
    }
    in_maps = []
    for c in range(NCORES):
        m = dict(common)
        m["idx16"] = cores[c]["idx16"]
        m["idxp"] = cores[c]["idxp"]
        m["rl"] = cores[c]["rl"]
        m["rbfT"] = cores[c]["rbfT"]
        in_maps.append(m)
    return in_maps


_CACHE = {}


def kernel(x, edge_index, edge_rbf, W1, b1, Wf1, bf1, Wf2, bf2, W2, b2):
    cores, tpbq, T, meta = _preprocess(edge_index, edge_rbf)
    key = (T, tuple(np.asarray(tpbq).ravel().tolist()))
    if key not in _CACHE:
        _CACHE[key] = _build_program(T, meta, reps=1)
    nc = _CACHE[key]
    in_maps = _make_in_maps(x, edge_index, edge_rbf, W1, b1, Wf1, bf1, Wf2,
                            bf2, W2, b2, cores, T)
    res = run_bass_kernel_spmd(nc, in_maps, list(range(NCORES)))
    out = np.concatenate([res.results[c]["out"] for c in range(NCORES)],
                         axis=0)
    return out.astype(np.float32)



# revision 14
# speedup vs baseline: 8.0911x; 4.9835x over previous
"""CFConv (SchNet continuous-filter conv) Bass kernel for 8 Trainium2 NeuronCores.

Strategy (graph/data parallel per the sharding hint):
- Nodes partitioned 12500/core; edges routed to the owner of their destination
  node (row) so the scatter-add is local to a core.
- Each core computes xw = x @ W1 + b1 for ALL nodes into an internal DRAM
  table (partition-striped: node k lives at row (k%128)*782 + k//128 of a
  [100096, 128] table, so phase-A writes are one descriptor per partition).
- Per-edge xw rows are fetched with gpsimd.dma_gather (the Q7 MoE gather:
  int16 indices wrapped in 16 partitions and replicated per Q7 core). int16
  limits a gather to a 25024-row window, so the table is split into 4
  quarters; an edge's quarter is (col%128)//32. Destination blocks of 128
  nodes are processed in supergroups of 4 blocks; edges are grouped
  (block, quarter) and padded to whole 128-edge tiles; one dma_gather per
  (supergroup, quarter) fetches all its tiles at once.
- Scatter-add is a matmul with a one-hot selection matrix S built on-device
  by an is_equal compare: outT[f, n] += msg[e, f].T @ S[e, n], accumulated in
  one PSUM bank per block (4 concurrent blocks).
- Filter MLP per up-to-4-tile window: h1T = Wf1.T @ rbfT (K=16), Silu(+bf1)
  on ACT, filt = hT_slice.T @ Wf2 per tile, msg = (filt + bf2) * xw_g on DVE.
- Per block: final = outT.T @ W2 + b2, DMA'd to the output rows.

SPMD: one program for all 8 cores; per-(core, block, quarter) edge counts are
padded to a common tile count (max over cores) so instruction streams match.
"""
import sys
sys.path.insert(0, '/opt/trn_rl_repo')
from contextlib import ExitStack

import ml_dtypes
import numpy as np

import concourse.bass as bass
import concourse.bacc as bacc
import concourse.tile as tile
from concourse import library_config, mybir
from concourse.bass_utils import run_bass_kernel_spmd

N_NODES = 100000
N_EDGES = 640000
D = 128
RBF = 16
NCORES = 8
NPC = N_NODES // NCORES        # 12500 nodes per core
BLK = 128
NBLK = (NPC + BLK - 1) // BLK  # 98 dest blocks per core
NTILES_X = (N_NODES + 127) // 128  # 782 node tiles in the xw table
NPAD = NTILES_X * 128          # 100096
NQ = 4
QROWS = NPAD // NQ             # 25024 table rows per quarter (int16-safe)
SGB = 4                        # blocks per supergroup
GRP = 4                        # edge tiles per filter-MLP window

F32 = mybir.dt.float32
BF16 = mybir.dt.bfloat16
I16 = mybir.dt.int16

assert QROWS == 32 * NTILES_X  # quarter of an edge == (col%128)//32


def _preprocess(edge_index, edge_rbf):
    """Route edges by dest owner; group by (dest block, table quarter); pad
    each group to whole 128-edge tiles with a common count across cores."""
    row = np.asarray(edge_index[0], dtype=np.int64)
    col = np.asarray(edge_index[1], dtype=np.int64)
    rbf = np.asarray(edge_rbf, dtype=np.float32)

    core = row // NPC
    per_core = []
    counts = np.zeros((NCORES, NBLK, NQ), dtype=np.int64)
    for c in range(NCORES):
        m = core == c
        r = row[m] - c * NPC
        cl = col[m]
        tr = (cl % 128) * NTILES_X + cl // 128   # striped table row
        # quarter-split so idx16 = tr - q*QROWS fits in int16 for dma_gather
        q = tr // QROWS
        b = r // BLK
        order = np.lexsort((q, b))
        r, cl, tr, q, b = r[order], cl[order], tr[order], q[order], b[order]
        rb = rbf[m][order]
        np.add.at(counts[c], (b, q), 1)
        per_core.append((r, tr, rb, b, q))

    tpbq = (counts.max(axis=0) + BLK - 1) // BLK          # [NBLK, NQ]
    for b in range(NBLK):
        if tpbq[b].sum() == 0:
            tpbq[b][0] = 1  # keep >=1 tile so outT is always written

    # tile stream: supergroups of SGB blocks; within one, quarter-major
    # (one dma_gather per (sg, q) needs its tiles contiguous)
    tiles = []          # (b, q) per tile
    ops_by_sg = []      # per supergroup: list of (q, tile_start, ntiles)
    for sg0 in range(0, NBLK, SGB):
        bs = range(sg0, min(sg0 + SGB, NBLK))
        sg_ops = []
        for q in range(NQ):
            nt = int(sum(tpbq[b][q] for b in bs))
            if nt == 0:
                continue
            sg_ops.append((q, len(tiles), nt))
            for b in bs:
                tiles.extend([(b, q)] * int(tpbq[b][q]))
        ops_by_sg.append(sg_ops)
    T = len(tiles)
    kmax = max(nt for sg in ops_by_sg for _, _, nt in sg)

    blk_first = {}
    blk_last = {}
    for t, (b, q) in enumerate(tiles):
        blk_first.setdefault(b, t)
        blk_last[b] = t

    # per-(b,q) tile start offsets in the stream
    seg_start = {}
    for t, (b, q) in enumerate(tiles):
        seg_start.setdefault((b, q), t)

    cores = []
    for c in range(NCORES):
        r, tr, rb, b, q = per_core[c]
        idx16 = np.zeros((T * BLK,), dtype=np.int16)
        idx32 = np.zeros((T * BLK,), dtype=np.int32)
        rl = np.full((T * BLK,), 255.0, dtype=np.float32)
        rbfT = np.zeros((RBF, T * BLK), dtype=np.float32)
        e0 = 0
        for bb in range(NBLK):
            for qq in range(NQ):
                n = int(counts[c, bb, qq])
                if n == 0:
                    continue
                d0 = seg_start[(bb, qq)] * BLK
                dst = slice(d0, d0 + n)
                idx16[dst] = (tr[e0:e0 + n] - qq * QROWS).astype(np.int16)
                idx32[dst] = tr[e0:e0 + n].astype(np.int32)
                rl[dst] = (r[e0:e0 + n] - bb * BLK).astype(np.float32)
                rbfT[:, dst] = rb[e0:e0 + n].T
                e0 += n
        # idx wrapped in 16 partitions, replicated for the 8 Q7 cores
        idxw = np.tile(idx16.reshape(T * 8, 16).T, (8, 1)).copy()
        cores.append({
            "idx16": idxw,                              # [128, T*8] int16
            "idxp": idx32.reshape(T, BLK).T.copy(),     # [128, T] int32
            "rl": rl.reshape(T, BLK).T.copy(),          # [128, T]
            "rbfT": rbfT,                               # [16, T*128]
        })
    meta = dict(tiles=tiles, ops_by_sg=ops_by_sg, kmax=int(kmax),
                blk_first=blk_first, blk_last=blk_last)
    return cores, tpbq, T, meta


def _build_program(T, meta, reps=1, sim_mode=False, do_phase_a=True,
                   do_phase_b=True, do_gather=True, do_compute=True,
                   use_dma_gather=True):
    tiles, ops_by_sg, kmax = meta["tiles"], meta["ops_by_sg"], meta["kmax"]
    blk_first, blk_last = meta["blk_first"], meta["blk_last"]

    nc = bacc.Bacc("TRN2", target_bir_lowering=False, debug=False,
                   num_devices=NCORES)

    xT_in = nc.dram_tensor("xT", [128, NPAD], F32, kind="ExternalInput").ap()
    W1_in = nc.dram_tensor("W1", [D, D], F32, kind="ExternalInput").ap()
    Wf1_in = nc.dram_tensor("Wf1", [RBF, D], F32, kind="ExternalInput").ap()
    Wf2_in = nc.dram_tensor("Wf2", [D, D], F32, kind="ExternalInput").ap()
    W2_in = nc.dram_tensor("W2", [D, D], F32, kind="ExternalInput").ap()
    b1bc_in = nc.dram_tensor("b1bc", [128, 4 * D], F32, kind="ExternalInput").ap()
    bf1c_in = nc.dram_tensor("bf1c", [128, 1], F32, kind="ExternalInput").ap()
    bf2bc_in = nc.dram_tensor("bf2bc", [128, D], F32, kind="ExternalInput").ap()
    b2bc_in = nc.dram_tensor("b2bc", [128, D], F32, kind="ExternalInput").ap()
    iota_in = nc.dram_tensor("iotar", [128, 128], F32, kind="ExternalInput").ap()
    idx_in = nc.dram_tensor("idx16", [128, T * 8], I16, kind="ExternalInput").ap()
    idxp_in = nc.dram_tensor("idxp", [128, T], mybir.dt.int32, kind="ExternalInput").ap()
    rl_in = nc.dram_tensor("rl", [128, T], F32, kind="ExternalInput").ap()
    rbfT_in = nc.dram_tensor("rbfT", [RBF, T * BLK], F32, kind="ExternalInput").ap()
    out_ext = nc.dram_tensor("out", [NPC, D], F32, kind="ExternalOutput").ap()

    # internal xw table, partition-striped: node k at [k%128, (k//128)*128+f].
    # Four naturally-shaped [QROWS, 128] quarter tensors (dma_gather wants a
    # plain [rows, elem] table); quarter q holds partitions [32q, 32q+32) of
    # the [128, NPAD] striped view. Writes go through a manual AP per quarter.
    xwq_h = [nc.dram_tensor(f"xw{q}", [QROWS, 128], F32, kind="ExternalOutput")
             for q in range(NQ)] if use_dma_gather else []
    xw_q = [h.ap() for h in xwq_h]
    # [32-partition, NPAD] write views (partition j of quarter q = global
    # partition 32q+j; its row-range is [j*NTILES_X, (j+1)*NTILES_X))
    xwq_w = [bass.AP(xw_q[q].tensor, 0, [[NPAD, 32], [1, NPAD]])
             for q in range(len(xw_q))]
    xw_tab = None  # indirect path keeps a single full-table tensor
    xw_h = nc.dram_tensor("xw", [128, NPAD], F32)
    xw_w = xw_h.ap()
    if not use_dma_gather:
        xw_tab = bass.AP(xw_w.tensor, 0, [[128, NPAD], [1, 128]])

    if not sim_mode:
        nc.gpsimd.load_library(library_config.mlp)

    XCH = 32
    NCH = (NTILES_X + XCH - 1) // XCH
    gsem = nc.alloc_semaphore("gsem")
    gcount = [0]

    with tile.TileContext(nc) as tc:
        with ExitStack() as ctx:
            res = ctx.enter_context(tc.tile_pool(name="res", bufs=1))
            xpool = ctx.enter_context(tc.tile_pool(name="xch", bufs=2))
            spool = ctx.enter_context(tc.tile_pool(name="stage", bufs=2))
            gpool = ctx.enter_context(tc.tile_pool(name="gath", bufs=2))
            rpool = ctx.enter_context(tc.tile_pool(name="rbfp", bufs=2))
            wpool = ctx.enter_context(tc.tile_pool(name="work", bufs=3))
            hpool = ctx.enter_context(tc.tile_pool(name="hts", bufs=3))
            opool = ctx.enter_context(tc.tile_pool(name="outs", bufs=2))
            pp_mm = ctx.enter_context(tc.tile_pool(name="psmm", bufs=2, space="PSUM"))
            pp_h1 = ctx.enter_context(tc.tile_pool(name="psh1", bufs=1, space="PSUM"))
            pp_out = ctx.enter_context(tc.tile_pool(name="psout", bufs=4, space="PSUM"))
            pp_fin = ctx.enter_context(tc.tile_pool(name="psfin", bufs=1, space="PSUM"))

            W1 = res.tile([D, D], F32); nc.sync.dma_start(W1[:], W1_in[:])
            Wf1 = res.tile([RBF, D], F32); nc.sync.dma_start(Wf1[:], Wf1_in[:])
            Wf2 = res.tile([D, D], F32); nc.sync.dma_start(Wf2[:], Wf2_in[:])
            W2 = res.tile([D, D], F32); nc.sync.dma_start(W2[:], W2_in[:])
            b1bc = res.tile([128, 4 * D], F32); nc.sync.dma_start(b1bc[:], b1bc_in[:])
            bf1c = res.tile([128, 1], F32); nc.sync.dma_start(bf1c[:], bf1c_in[:])
            bf2bc = res.tile([128, D], F32); nc.sync.dma_start(bf2bc[:], bf2bc_in[:])
            b2bc = res.tile([128, D], F32); nc.sync.dma_start(b2bc[:], b2bc_in[:])
            iotar = res.tile([128, 128], F32); nc.sync.dma_start(iotar[:], iota_in[:])
            idx_sb = res.tile([128, T * 8], I16); nc.sync.dma_start(idx_sb[:], idx_in[:])
            idxp = res.tile([128, T], mybir.dt.int32); nc.sync.dma_start(idxp[:], idxp_in[:])
            rl = res.tile([128, T], F32); nc.sync.dma_start(rl[:], rl_in[:])
            # token tile: every gather critical-unit writes it, forcing Tile
            # to keep the units in emission order on the gpsimd stream (the
            # cumulative gsem waits rely on that order). xwtap is a dummy
            # Tile-visible read of the xw tensor for phase-A -> B ordering.
            token = res.tile([1, 8], F32)
            xwtap = res.tile([1, 8], F32)

            for _rep in range(reps):
                # ---- phase A: xw = x @ W1 + b1 for all nodes ----
                for ch in range(NCH if do_phase_a else 0):
                    i0 = ch * XCH
                    nt = min(XCH, NTILES_X - i0)
                    xch = xpool.tile([128, XCH * 128], F32, tag="xch")
                    nc.sync.dma_start(xch[:, :nt * 128],
                                      xT_in[:, i0 * 128:(i0 + nt) * 128])
                    stage = spool.tile([128, XCH * 128], F32, tag="stage")
                    for i0q in range(0, nt, 4):
                        nq = min(4, nt - i0q)
                        xw_ps = pp_h1.tile([128, GRP * BLK], F32, tag="h1",
                                           space="PSUM", name="xwA")
                        for i in range(i0q, i0q + nq):
                            nc.tensor.matmul(
                                xw_ps[:, (i - i0q) * 128:(i - i0q + 1) * 128],
                                lhsT=xch[:, i * 128:(i + 1) * 128],
                                rhs=W1[:], start=True, stop=True)
                        nc.vector.tensor_add(
                            stage[:, i0q * 128:(i0q + nq) * 128],
                            xw_ps[:, :nq * 128], b1bc[:, :nq * 128])
                    if use_dma_gather:
                        c0, c1 = i0 * 128, (i0 + nt) * 128
                        for q in range(NQ):
                            wv = bass.AP(xw_q[q].tensor, c0,
                                         [[NPAD, 32], [1, c1 - c0]])
                            nc.sync.dma_start(wv,
                                              stage[32 * q:32 * (q + 1),
                                                    :nt * 128])
                    else:
                        nc.sync.dma_start(xw_w[:, i0 * 128:(i0 + nt) * 128],
                                          stage[:, :nt * 128])

                # ---- phase B: edges, one gather per (supergroup, quarter) ----
                if do_phase_b:
                    outT = {}
                    for sgi, sg0 in enumerate(range(0, NBLK, SGB)):
                        bs = list(range(sg0, min(sg0 + SGB, NBLK)))
                        for b in bs:
                            outT[b] = pp_out.tile([128, 128], F32, tag="outT",
                                                  space="PSUM", name=f"oT{b}")
                        for q, t0, ntq in ops_by_sg[sgi]:
                            xwg = None
                            if do_gather or do_compute:
                                xwg = gpool.tile([128, kmax, 128], F32,
                                                 tag="xwg", name="xwg")
                            if do_gather and use_dma_gather:
                                # the SWDGE descriptor ring holds 1024
                                # descriptors; one dma_gather op must stay
                                # <= 1024 indices (8 tiles) or the Q7
                                # handler crashes the NEFF.
                                for c0 in range(0, ntq, 8):
                                    csz = min(8, ntq - c0)
                                    nc.gpsimd.dma_gather(
                                        out_ap=xwg[:, c0:c0 + csz, :],
                                        in_ap=xw_q[q],
                                        idxs_ap=idx_sb[:, (t0 + c0) * 8:
                                                       (t0 + c0 + csz) * 8],
                                        num_idxs=csz * BLK,
                                        num_idxs_reg=csz * BLK,
                                        elem_size=D)
                            rbft = rpool.tile([RBF, kmax * BLK], F32, tag="rbf")
                            nc.sync.dma_start(
                                rbft[:, :ntq * BLK],
                                rbfT_in[:, t0 * BLK:(t0 + ntq) * BLK])
                            if not do_compute:
                                continue
                            for g0 in range(0, ntq, GRP):
                                gsz = min(GRP, ntq - g0)
                                h1 = pp_h1.tile([128, GRP * BLK], F32,
                                                tag="h1", space="PSUM")
                                nc.tensor.matmul(
                                    h1[:, :gsz * BLK], lhsT=Wf1[:],
                                    rhs=rbft[:, g0 * BLK:(g0 + gsz) * BLK],
                                    start=True, stop=True)
                                hT = hpool.tile([128, GRP * BLK], F32, tag="hT")
                                if sim_mode:
                                    sg_t = hpool.tile([128, GRP * BLK], F32,
                                                      tag="sg")
                                    nc.scalar.activation(
                                        sg_t[:, :gsz * BLK], h1[:, :gsz * BLK],
                                        mybir.ActivationFunctionType.Sigmoid,
                                        bias=bf1c[:, :1], scale=1.0)
                                    zz = hpool.tile([128, GRP * BLK], F32,
                                                    tag="zz")
                                    nc.vector.tensor_scalar(
                                        out=zz[:, :gsz * BLK],
                                        in0=h1[:, :gsz * BLK],
                                        scalar1=bf1c[:, :1], scalar2=None,
                                        op0=mybir.AluOpType.add)
                                    nc.vector.tensor_mul(hT[:, :gsz * BLK],
                                                         zz[:, :gsz * BLK],
                                                         sg_t[:, :gsz * BLK])
                                else:
                                    nc.scalar.activation(
                                        hT[:, :gsz * BLK], h1[:, :gsz * BLK],
                                        mybir.ActivationFunctionType.Silu,
                                        bias=bf1c[:, :1], scale=1.0)
                                for i in range(gsz):
                                    t = t0 + g0 + i
                                    b = tiles[t][0]
                                    if not use_dma_gather and do_gather:
                                        xwg_t = gpool.tile([128, 128], F32,
                                                           tag="xwgt")
                                        nc.gpsimd.indirect_dma_start(
                                            out=xwg_t[:], out_offset=None,
                                            in_=xw_tab,
                                            in_offset=bass.IndirectOffsetOnAxis(
                                                ap=idxp[:, t:t + 1], axis=0))
                                        xsrc = xwg_t[:]
                                    else:
                                        xsrc = xwg[:, g0 + i, :]
                                    S = wpool.tile([128, 128], F32, tag="S")
                                    nc.vector.tensor_tensor(
                                        out=S[:], in0=iotar[:],
                                        in1=rl[:, t:t + 1].to_broadcast(
                                            [128, 128]),
                                        op=mybir.AluOpType.is_equal)
                                    filt_ps = pp_mm.tile([128, 128], F32,
                                                         tag="mm128",
                                                         space="PSUM")
                                    nc.tensor.matmul(
                                        filt_ps[:],
                                        lhsT=hT[:, i * 128:(i + 1) * 128],
                                        rhs=Wf2[:], start=True, stop=True)
                                    msg = wpool.tile([128, 128], F32, tag="msg")
                                    nc.vector.scalar_tensor_tensor(
                                        out=msg[:], in0=filt_ps[:], scalar=1.0,
                                        in1=bf2bc[:], op0=mybir.AluOpType.mult,
                                        op1=mybir.AluOpType.add)
                                    msgm = wpool.tile([128, 128], F32,
                                                      tag="msgm")
                                    nc.vector.tensor_mul(msgm[:], msg[:],
                                                         xsrc)
                                    nc.tensor.matmul(
                                        outT[b][:], lhsT=msgm[:], rhs=S[:],
                                        start=(t == blk_first[b]),
                                        stop=(t == blk_last[b]))
                        if not do_compute:
                            continue
                        for b in bs:
                            outT_sb = opool.tile([128, 128], F32, tag="outTsb")
                            nc.vector.tensor_copy(outT_sb[:], outT[b][:])
                            fin_ps = pp_fin.tile([128, 128], F32, tag="fin",
                                                 space="PSUM")
                            nc.tensor.matmul(fin_ps[:], lhsT=outT_sb[:],
                                             rhs=W2[:], start=True, stop=True)
                            fin = opool.tile([128, 128], F32, tag="fin_sb")
                            nc.vector.tensor_add(fin[:], fin_ps[:], b2bc[:])
                            rows = min(BLK, NPC - b * BLK)
                            nc.sync.dma_start(
                                out_ext[b * BLK:b * BLK + rows, :],
                                fin[:rows, :])
    nc.compile()
    return nc


def _make_in_maps(x, edge_index, edge_rbf, W1, b1, Wf1, bf1, Wf2, bf2, W2, b2,
                  cores, T):
    xT = np.zeros((128, NPAD), dtype=np.float32)
    xp = np.zeros((NPAD, D), dtype=np.float32)
    xp[:N_NODES] = np.asarray(x, dtype=np.float32)
    # xT[:, i*128:(i+1)*128] is node-tile i, feature-on-partition
    xT[:] = xp.reshape(NTILES_X, 128, D).transpose(2, 0, 1).reshape(D, NPAD)

    common = {
        "xT": xT,
        "W1": np.asarray(W1, np.float32),
        "Wf1": np.asarray(Wf1, np.float32),
        "Wf2": np.asarray(Wf2, np.float32),
        "W2": np.asarray(W2, np.float32),
        "b1bc": np.tile(np.broadcast_to(np.asarray(b1, np.float32),
                                        (128, D)), (1, 4)).copy(),
        "bf1c": np.asarray(bf1, np.float32).reshape(128, 1).copy(),
        "bf2bc": np.broadcast_to(np.asarray(bf2, np.float32), (128, D)).copy(),
        "b2bc": np.broadcast_to(np.asarray(b2, np.float32), (128, D)).copy(),
        "iotar": np.broadcast_to(np.arange(128, dtype=np.float32),
                                 (128, 128)).copy(),
    }
    in_maps = []
    for c in range(NCORES):
        m = dict(common)
        m["idx16"] = cores[c]["idx16"]
        m["idxp"] = cores[c]["idxp"]
        m["rl"] = cores[c]["rl"]
        m["rbfT"] = cores[c]["rbfT"]
        in_maps.append(m)
    return in_maps


_CACHE = {}


def kernel(x, edge_index, edge_rbf, W1, b1, Wf1, bf1, Wf2, bf2, W2, b2):
    cores, tpbq, T, meta = _preprocess(edge_index, edge_rbf)
    key = (T, tuple(np.asarray(tpbq).ravel().tolist()))
    if key not in _CACHE:
        _CACHE[key] = _build_program(T, meta, reps=1)
    nc = _CACHE[key]
    in_maps = _make_in_maps(x, edge_index, edge_rbf, W1, b1, Wf1, bf1, Wf2,
                            bf2, W2, b2, cores, T)
    res = run_bass_kernel_spmd(nc, in_maps, list(range(NCORES)))
    out = np.concatenate([res.results[c]["out"] for c in range(NCORES)],
                         axis=0)
    return out.astype(np.float32)

